# revision 8
# baseline (speedup 1.0000x reference)
"""Gemma4 MoE feed-forward on 8 Trainium2 NeuronCores.

Strategy: expert-parallel. E == n_cores == 8, so core e owns expert e's
weights (Wg[e], Wu[e], Wd[e]) and receives exactly the tokens routed to
expert e (gathered + transposed + padded on the host). Each core runs a
dense gated-FFN over its token batch:

    dT = Wd^T @ (gelu_tanh(Wg^T x^T) * (Wu^T x^T))        (all [*, C] layouts)

The host then scatter-adds routing_weight * dT^T back into the full
[T, H] output. Tokens that select the same expert in both slots are
deduplicated on the host (weights summed).

Matmul inputs are bf16 (full PE rate, rel err ~4e-3 on this problem,
half the HBM traffic of fp32); PSUM accumulation and the output stay
fp32.

Per-DMA-instruction cost on the issuing engine is ~625 ns (HWDGE
descriptor-gen), so the kernel keeps the DMA count low: one transfer
per weight tensor per k-tile (4 KB/partition contiguous, host
pre-tiled) and one full-width transfer per x row-tile, split across
the Sync and Scalar HWDGE queues so enqueue is never the serializer.
"""

import os
import sys

import numpy as np

for _p in ("/opt/trn_rl_repo", "/root/.axon_site/_ro/trn_rl_repo"):
    if os.path.isdir(_p) and _p not in sys.path:
        sys.path.append(_p)

T, H, I, E, K = 4096, 2048, 1024, 8, 2
NCORES = 8
KH = H // 128  # 16 k-tiles over the hidden dim
KI = I // 128  # 8 k-tiles over the intermediate dim

_PROGRAM_CACHE = {}
LAST_RESULT = None  # BassKernelResults of the most recent run (for test.py)
TRACE = False  # test.py sets this to capture an NTFF profile
TRACE_CORES = [0]

WARMUP_MM = 8  # dummy matmuls at launch: HAM clock ramp + fill DMA lead-in
XG = 4  # k-tiles per x DMA group


def _tile_w_up(W, bf16):
    """[H, I] -> [KI, 128, KH*128]: tile[i, p, k*128+j] = W[k*128+p, i*128+j],
    so expert i-tile i loads as ONE dma with 4KB/partition contiguous."""
    Wt = W.reshape(KH, 128, KI, 128).transpose(2, 1, 0, 3)
    return np.ascontiguousarray(Wt, dtype=bf16).reshape(KI, 128, KH * 128)


def _tile_w_down(W, bf16):
    """[I, H] -> [KH, 128, KI*128] (same scheme, contraction over I)."""
    Wt = W.reshape(KI, 128, KH, 128).transpose(2, 1, 0, 3)
    return np.ascontiguousarray(Wt, dtype=bf16).reshape(KH, 128, KI * 128)


def _pick_config(max_count):
    """Token-block config: NT blocks of even width N, NT*N >= max_count,
    N <= 512 (PSUM bank limit)."""
    mc = max(max_count, 128)
    nt = -(-mc // 512)
    n = -(-mc // nt)
    n += n % 2
    return (nt * n, nt, n)  # (C, NT, N)


def _build_program(C, NT, N):
    import concourse.tile as tile
    from concourse import bacc, mybir
    from contextlib import ExitStack

    f32 = mybir.dt.float32
    bf16 = mybir.dt.bfloat16

    nc = bacc.Bacc("TRN2", target_bir_lowering=False, debug=False)

    xT = nc.dram_tensor("xT", [H, C], bf16, kind="ExternalInput").ap()
    Wg_d = nc.dram_tensor("Wg", [KI, 128, KH * 128], bf16, kind="ExternalInput").ap()
    Wu_d = nc.dram_tensor("Wu", [KI, 128, KH * 128], bf16, kind="ExternalInput").ap()
    Wd_d = nc.dram_tensor("Wd", [KH, 128, KI * 128], bf16, kind="ExternalInput").ap()
    dT = nc.dram_tensor("dT", [H, C], f32, kind="ExternalOutput").ap()

    # Partition-major views: row a*128+p -> partition p, free index a.
    xT_p = xT.rearrange("(a p) c -> p a c", p=128)  # [128, KH, C]
    dT_p = dT.rearrange("(a p) c -> p a c", p=128)  # [128, KH, C]

    GELU = mybir.ActivationFunctionType.Gelu_apprx_tanh

    with tile.TileContext(nc) as tc, ExitStack() as ctx:
        xpool = ctx.enter_context(tc.tile_pool(name="x", bufs=1))
        wpool = ctx.enter_context(tc.tile_pool(name="w", bufs=3))
        wdpool = ctx.enter_context(tc.tile_pool(name="wd", bufs=1))
        apool = ctx.enter_context(tc.tile_pool(name="a", bufs=1))
        tpool = ctx.enter_context(tc.tile_pool(name="t", bufs=4))
        opool = ctx.enter_context(tc.tile_pool(name="o", bufs=4))

        # PE clock-gate warmup: HAM starts at 1.2 GHz and un-throttles only
        # after ~3.4us of sustained activity; dummy matmuls on memset scratch
        # also bridge the first weight/x DMA latency.
        with (
            tc.tile_pool(name="warm", bufs=1) as wmpool,
            tc.tile_pool(name="warmps", bufs=1, space="PSUM") as wmpspool,
        ):
            wt = wmpool.tile([128, 512], bf16, name="warm_in")
            nc.gpsimd.memset(wt[:], 0.0)
            wps = wmpspool.tile([128, 512], f32, name="warm_ps")
            for r in range(WARMUP_MM):
                nc.tensor.matmul(wps[:], wt[:, 0:128], wt[:], start=True, stop=True)

        w_tiles = {}

        def issue_w(i, split=False):
            wgt = wpool.tile([128, KH * 128], bf16, tag="wg", name=f"wg{i}")
            wut = wpool.tile([128, KH * 128], bf16, tag="wu", name=f"wu{i}")
            if split:
                # i=0 only: interleave k-halves of wg/wu so the PE's earliest
                # needs land first under head DMA-bandwidth contention.
                hh = KH * 128 // 2
                nc.sync.dma_start(wgt[:, 0:hh], Wg_d[i, :, 0:hh])
                nc.sync.dma_start(wut[:, 0:hh], Wu_d[i, :, 0:hh])
                nc.sync.dma_start(wgt[:, hh:], Wg_d[i, :, hh:])
                nc.sync.dma_start(wut[:, hh:], Wu_d[i, :, hh:])
            else:
                nc.sync.dma_start(wgt[:], Wg_d[i])
                nc.sync.dma_start(wut[:], Wu_d[i])
            w_tiles[i] = (wgt, wut)

        wd_tiles = {}

        def issue_wd(h):
            wdt = wdpool.tile([128, KI * 128], bf16, tag=f"wd{h}", name=f"wd{h}")
            nc.sync.dma_start(wdt[:], Wd_d[h])
            wd_tiles[h] = wdt

        # Head issue order, matched to PE consumption under fair-share DMA
        # bandwidth between the two HWDGE queues: i=0 weights in k-halves on
        # Sync, x in XG-k-tile groups n=0 column-half first, spread over both
        # queues so each queue's byte load matches when the PE needs it.
        NXG = KH // XG
        issue_w(0, split=True)
        xgs = [xpool.tile([128, XG, C], bf16, name=f"xg{j}") for j in range(NXG)]

        def issue_x(eng, j, n):
            eng.dma_start(
                xgs[j][:, :, n * N : (n + 1) * N],
                xT_p[:, j * XG : (j + 1) * XG, n * N : (n + 1) * N],
            )

        for j in range(NXG - 1):
            issue_x(nc.scalar, j, 0)
        issue_x(nc.sync, NXG - 1, 0)
        for n in range(1, NT):
            for j in range(NXG):
                issue_x(nc.sync if j < NXG // 2 else nc.scalar, j, n)
        issue_w(1)

        def xk(k, nsl):
            return xgs[k // XG][:, k % XG, nsl]

        aT = apool.tile([128, KI, C], bf16, name="aT")

        # All 16 down-weight fetches are spread over the up phase, so the
        # down phase starts with every Wd tile resident.
        wd_sched = {2: [0, 1, 2], 3: [3, 4, 5], 4: [6, 7, 8], 5: [9, 10, 11],
                    6: [12, 13, 14], 7: [15]}

        d_bufs = 2 if 4 + 2 * NT <= 8 else 1
        with (
            tc.tile_pool(name="gu", bufs=2, space="PSUM") as gupool,
            tc.tile_pool(name="d", bufs=d_bufs, space="PSUM") as dpool,
        ):
            for i in range(KI):
                if i + 2 < KI:
                    issue_w(i + 2)
                for h in wd_sched.get(i, []):
                    issue_wd(h)
                wgt, wut = w_tiles.pop(i)
                for n in range(NT):
                    nsl = slice(n * N, (n + 1) * N)
                    g_ps = gupool.tile([128, N], f32, tag="g", name=f"g{i}_{n}")
                    u_ps = gupool.tile([128, N], f32, tag="u", name=f"u{i}_{n}")
                    for k in range(KH):
                        ksl = slice(k * 128, (k + 1) * 128)
                        nc.tensor.matmul(
                            g_ps[:], wgt[:, ksl], xk(k, nsl),
                            start=(k == 0), stop=(k == KH - 1),
                        )
                        nc.tensor.matmul(
                            u_ps[:], wut[:, ksl], xk(k, nsl),
                            start=(k == 0), stop=(k == KH - 1),
                        )
                    gel = tpool.tile([128, N], f32, tag="gelu", name=f"gel{i}_{n}")
                    nc.scalar.activation(gel[:], g_ps[:], GELU)
                    nc.vector.tensor_mul(aT[:, i, nsl], gel[:], u_ps[:])

            for h in range(KH):
                if h not in wd_tiles:
                    issue_wd(h)
                wdt = wd_tiles.pop(h)
                if h < KH - 1:
                    d_ps = [
                        dpool.tile([128, N], f32, tag=f"d{n}", name=f"d{h}_{n}")
                        for n in range(NT)
                    ]
                    for ki in range(KI):
                        lw = wdt[:, ki * 128 : (ki + 1) * 128]
                        for n in range(NT):
                            nc.tensor.matmul(
                                d_ps[n][:], lw, aT[:, ki, n * N : (n + 1) * N],
                                start=(ki == 0), stop=(ki == KI - 1),
                            )
                    for n in range(NT):
                        o = opool.tile([128, N], f32, tag="o", name=f"o{h}_{n}")
                        nc.vector.tensor_copy(o[:], d_ps[n][:])
                        eng = nc.sync if n % 2 == 0 else nc.scalar
                        eng.dma_start(dT_p[:, h, n * N : (n + 1) * N], o[:])
                else:
                    # Last h: finish the n-chains one at a time and split each
                    # output over both HWDGE queues by partition halves, so
                    # the tail after the final matmul is one short transfer.
                    for n in range(NT):
                        nsl = slice(n * N, (n + 1) * N)
                        d_ps = dpool.tile([128, N], f32, tag=f"d{n}", name=f"d{h}_{n}")
                        for ki in range(KI):
                            nc.tensor.matmul(
                                d_ps[:], wdt[:, ki * 128 : (ki + 1) * 128],
                                aT[:, ki, nsl],
                                start=(ki == 0), stop=(ki == KI - 1),
                            )
                        o = opool.tile([128, N], f32, tag="o", name=f"o{h}_{n}")
                        nc.vector.tensor_copy(o[:], d_ps[:])
                        nc.sync.dma_start(dT_p[0:64, h, nsl], o[0:64, :])
                        nc.scalar.dma_start(dT_p[64:128, h, nsl], o[64:128, :])

    nc.compile()
    return nc


def _get_program(C, NT, N):
    key = (C, NT, N)
    if key not in _PROGRAM_CACHE:
        _PROGRAM_CACHE[key] = _build_program(C, NT, N)
    return _PROGRAM_CACHE[key]


def _ensure_ntff_hook():
    """Register the axon NTFF profile hook if the image's antenv lacks
    axon_hooks (see trn_agent_boot.trn_boot). Only needed when TRACE."""
    import types

    try:
        from antenv.axon_hooks import get_axon_ntff_profile_hook  # noqa: F401

        return
    except ImportError:
        pass
    import antenv
    from trn_agent_boot.trn_boot import _ntff_profile_via_ctypes

    hook = _ntff_profile_via_ctypes("/opt/axon/libaxon_pjrt.so")
    mod = types.ModuleType("antenv.axon_hooks")
    state = {"hook": hook}
    mod.set_axon_ntff_profile_hook = lambda h: state.__setitem__("hook", h)
    mod.get_axon_ntff_profile_hook = lambda: state["hook"]
    sys.modules["antenv.axon_hooks"] = mod
    antenv.axon_hooks = mod


def kernel(x, Wg, Wu, Wd, selected_experts, routing_weights):
    global LAST_RESULT
    import ml_dtypes
    from concourse.bass_utils import run_bass_kernel_spmd

    if TRACE:
        _ensure_ntff_hook()

    bf16 = ml_dtypes.bfloat16

    x = np.asarray(x, dtype=np.float32)
    Wg = np.asarray(Wg, dtype=np.float32)
    Wu = np.asarray(Wu, dtype=np.float32)
    Wd = np.asarray(Wd, dtype=np.float32)
    selected_experts = np.asarray(selected_experts)
    routing_weights = np.asarray(routing_weights, dtype=np.float32)

    # Host-side dispatch: per expert, the (deduplicated) token list and
    # summed routing weights.
    idx_list, w_list = [], []
    for e in range(E):
        m = selected_experts == e  # [T, K]
        idx = np.nonzero(m.any(axis=1))[0]
        w = (routing_weights * m).sum(axis=1)[idx]
        idx_list.append(idx)
        w_list.append(w.astype(np.float32))

    max_count = max(len(idx) for idx in idx_list)
    C, NT, N = _pick_config(max_count)

    nc = _get_program(C, NT, N)

    in_maps = []
    for e in range(E):
        idx = idx_list[e]
        xT = np.zeros((H, C), dtype=bf16)
        xT[:, : len(idx)] = np.ascontiguousarray(x[idx].T, dtype=bf16)
        in_maps.append(
            {
                "xT": xT,
                "Wg": _tile_w_up(Wg[e], bf16),
                "Wu": _tile_w_up(Wu[e], bf16),
                "Wd": _tile_w_down(Wd[e], bf16),
            }
        )

    res = run_bass_kernel_spmd(
        nc,
        in_maps,
        list(range(NCORES)),
        trace=TRACE,
        trace_cores=TRACE_CORES if TRACE else None,
    )
    LAST_RESULT = res

    out = np.zeros((T, H), dtype=np.float32)
    for e in range(E):
        idx = idx_list[e]
        dTe = res.results[e]["dT"]  # [H, C] fp32
        out[idx] += w_list[e][:, None] * dTe[:, : len(idx)].T
    return out


# revision 9
# speedup vs baseline: 1.1674x; 1.1674x over previous
"""Gemma4 MoE feed-forward on 8 Trainium2 NeuronCores.

Strategy: expert-parallel. E == n_cores == 8, so core e owns expert e's
weights (Wg[e], Wu[e], Wd[e]) and receives exactly the tokens routed to
expert e (gathered + transposed + padded on the host). Each core runs a
dense gated-FFN over its token batch:

    dT = Wd^T @ (gelu_tanh(Wg^T x^T) * (Wu^T x^T))        (all [*, C] layouts)

The host then scatter-adds routing_weight * dT^T back into the full
[T, H] output. Tokens that select the same expert in both slots are
deduplicated on the host (weights summed).

Matmul inputs are bf16 (full PE rate, rel err ~4e-3 on this problem,
half the HBM traffic of fp32); PSUM accumulation and the output stay
fp32.

Per-DMA-instruction cost on the issuing engine is ~625 ns (HWDGE
descriptor-gen), so the kernel keeps the DMA count low: one transfer
per weight tensor per k-tile (4 KB/partition contiguous, host
pre-tiled) and one full-width transfer per x row-tile, split across
the Sync and Scalar HWDGE queues so enqueue is never the serializer.
"""

import os
import sys

import numpy as np

for _p in ("/opt/trn_rl_repo", "/root/.axon_site/_ro/trn_rl_repo"):
    if os.path.isdir(_p) and _p not in sys.path:
        sys.path.append(_p)

T, H, I, E, K = 4096, 2048, 1024, 8, 2
NCORES = 8
KH = H // 128  # 16 k-tiles over the hidden dim
KI = I // 128  # 8 k-tiles over the intermediate dim

_PROGRAM_CACHE = {}
LAST_RESULT = None  # BassKernelResults of the most recent run (for test.py)
TRACE = False  # test.py sets this to capture an NTFF profile
TRACE_CORES = [0]

# Dummy matmuls at launch: HAM starts ~1.2 GHz and promotes to 2.4 only
# after ~6us of sustained PE activity, so the warmup must run to completion
# before the first data-gated stall. 8 was measured to leave the clock
# stuck at ~2.0 GHz for the WHOLE kernel; 12 reaches 2.4.
WARMUP_MM = 12
XG = 4  # k-tiles per x DMA group


def _tile_w_up(W, bf16):
    """[H, I] -> [KI, 128, KH*128]: tile[i, p, k*128+j] = W[k*128+p, i*128+j],
    so expert i-tile i loads as ONE dma with 4KB/partition contiguous."""
    Wt = W.reshape(KH, 128, KI, 128).transpose(2, 1, 0, 3)
    return np.ascontiguousarray(Wt, dtype=bf16).reshape(KI, 128, KH * 128)


def _tile_w_down(W, bf16):
    """[I, H] -> [KH, 128, KI*128] (same scheme, contraction over I)."""
    Wt = W.reshape(KI, 128, KH, 128).transpose(2, 1, 0, 3)
    return np.ascontiguousarray(Wt, dtype=bf16).reshape(KH, 128, KI * 128)


def _pick_config(max_count):
    """Token-block config: NT blocks of even width N, NT*N >= max_count,
    N <= 512 (PSUM bank limit)."""
    mc = max(max_count, 128)
    nt = -(-mc // 512)
    n = -(-mc // nt)
    n += n % 2
    return (nt * n, nt, n)  # (C, NT, N)


def _build_program(C, NT, N):
    import concourse.tile as tile
    from concourse import bacc, mybir
    from contextlib import ExitStack

    f32 = mybir.dt.float32
    bf16 = mybir.dt.bfloat16

    nc = bacc.Bacc("TRN2", target_bir_lowering=False, debug=False)

    xT = nc.dram_tensor("xT", [H, C], bf16, kind="ExternalInput").ap()
    Wg_d = nc.dram_tensor("Wg", [KI, 128, KH * 128], bf16, kind="ExternalInput").ap()
    Wu_d = nc.dram_tensor("Wu", [KI, 128, KH * 128], bf16, kind="ExternalInput").ap()
    Wd_d = nc.dram_tensor("Wd", [KH, 128, KI * 128], bf16, kind="ExternalInput").ap()
    dT = nc.dram_tensor("dT", [H, C], f32, kind="ExternalOutput").ap()

    # Partition-major views: row a*128+p -> partition p, free index a.
    xT_p = xT.rearrange("(a p) c -> p a c", p=128)  # [128, KH, C]
    dT_p = dT.rearrange("(a p) c -> p a c", p=128)  # [128, KH, C]

    GELU = mybir.ActivationFunctionType.Gelu_apprx_tanh

    with tile.TileContext(nc) as tc, ExitStack() as ctx:
        xpool = ctx.enter_context(tc.tile_pool(name="x", bufs=1))
        wpool = ctx.enter_context(tc.tile_pool(name="w", bufs=3))
        wdpool = ctx.enter_context(tc.tile_pool(name="wd", bufs=1))
        apool = ctx.enter_context(tc.tile_pool(name="a", bufs=1))
        tpool = ctx.enter_context(tc.tile_pool(name="t", bufs=4))
        opool = ctx.enter_context(tc.tile_pool(name="o", bufs=4))

        # PE clock-gate warmup: HAM starts at 1.2 GHz and un-throttles only
        # after ~3.4us of sustained activity; dummy matmuls on memset scratch
        # also bridge the first weight/x DMA latency.
        with (
            tc.tile_pool(name="warm", bufs=1) as wmpool,
            tc.tile_pool(name="warmps", bufs=1, space="PSUM") as wmpspool,
        ):
            wt = wmpool.tile([128, 512], bf16, name="warm_in")
            nc.gpsimd.memset(wt[:], 0.0)
            wps = wmpspool.tile([128, 512], f32, name="warm_ps")
            for r in range(WARMUP_MM):
                nc.tensor.matmul(wps[:], wt[:, 0:128], wt[:], start=True, stop=True)

        w_tiles = {}

        def issue_w(i, split=False):
            wgt = wpool.tile([128, KH * 128], bf16, tag="wg", name=f"wg{i}")
            wut = wpool.tile([128, KH * 128], bf16, tag="wu", name=f"wu{i}")
            if split:
                # i=0 only: interleave k-halves of wg/wu so the PE's earliest
                # needs land first under head DMA-bandwidth contention.
                hh = KH * 128 // 2
                nc.sync.dma_start(wgt[:, 0:hh], Wg_d[i, :, 0:hh])
                nc.sync.dma_start(wut[:, 0:hh], Wu_d[i, :, 0:hh])
                nc.sync.dma_start(wgt[:, hh:], Wg_d[i, :, hh:])
                nc.sync.dma_start(wut[:, hh:], Wu_d[i, :, hh:])
            else:
                nc.sync.dma_start(wgt[:], Wg_d[i])
                nc.sync.dma_start(wut[:], Wu_d[i])
            w_tiles[i] = (wgt, wut)

        wd_tiles = {}

        def issue_wd(h):
            wdt = wdpool.tile([128, KI * 128], bf16, tag=f"wd{h}", name=f"wd{h}")
            nc.sync.dma_start(wdt[:], Wd_d[h])
            wd_tiles[h] = wdt

        # Head issue order, matched to PE consumption under fair-share DMA
        # bandwidth between the two HWDGE queues: i=0 weights in k-halves on
        # Sync, x in XG-k-tile groups n=0 column-half first, spread over both
        # queues so each queue's byte load matches when the PE needs it.
        NXG = KH // XG
        issue_w(0, split=True)
        xgs = [xpool.tile([128, XG, C], bf16, name=f"xg{j}") for j in range(NXG)]

        def issue_x(eng, j, n):
            eng.dma_start(
                xgs[j][:, :, n * N : (n + 1) * N],
                xT_p[:, j * XG : (j + 1) * XG, n * N : (n + 1) * N],
            )

        for j in range(NXG - 1):
            issue_x(nc.scalar, j, 0)
        issue_x(nc.sync, NXG - 1, 0)
        for n in range(1, NT):
            for j in range(NXG):
                issue_x(nc.sync if j < NXG // 2 else nc.scalar, j, n)
        issue_w(1)

        def xk(k, nsl):
            return xgs[k // XG][:, k % XG, nsl]

        aT = apool.tile([128, KI, C], bf16, name="aT")

        # All 16 down-weight fetches are spread over the up phase, so the
        # down phase starts with every Wd tile resident.
        wd_sched = {2: [0, 1, 2], 3: [3, 4, 5], 4: [6, 7, 8], 5: [9, 10, 11],
                    6: [12, 13, 14], 7: [15]}

        d_bufs = 2 if 4 + 2 * NT <= 8 else 1
        with (
            tc.tile_pool(name="gu", bufs=2, space="PSUM") as gupool,
            tc.tile_pool(name="d", bufs=d_bufs, space="PSUM") as dpool,
        ):
            for i in range(KI):
                if i + 2 < KI:
                    issue_w(i + 2)
                for h in wd_sched.get(i, []):
                    issue_wd(h)
                wgt, wut = w_tiles.pop(i)
                for n in range(NT):
                    nsl = slice(n * N, (n + 1) * N)
                    g_ps = gupool.tile([128, N], f32, tag="g", name=f"g{i}_{n}")
                    u_ps = gupool.tile([128, N], f32, tag="u", name=f"u{i}_{n}")
                    for k in range(KH):
                        ksl = slice(k * 128, (k + 1) * 128)
                        nc.tensor.matmul(
                            g_ps[:], wgt[:, ksl], xk(k, nsl),
                            start=(k == 0), stop=(k == KH - 1),
                        )
                        nc.tensor.matmul(
                            u_ps[:], wut[:, ksl], xk(k, nsl),
                            start=(k == 0), stop=(k == KH - 1),
                        )
                    gel = tpool.tile([128, N], f32, tag="gelu", name=f"gel{i}_{n}")
                    nc.scalar.activation(gel[:], g_ps[:], GELU)
                    nc.vector.tensor_mul(aT[:, i, nsl], gel[:], u_ps[:])

            for h in range(KH):
                if h not in wd_tiles:
                    issue_wd(h)
                wdt = wd_tiles.pop(h)
                if h < KH - 1:
                    d_ps = [
                        dpool.tile([128, N], f32, tag=f"d{n}", name=f"d{h}_{n}")
                        for n in range(NT)
                    ]
                    for ki in range(KI):
                        lw = wdt[:, ki * 128 : (ki + 1) * 128]
                        for n in range(NT):
                            nc.tensor.matmul(
                                d_ps[n][:], lw, aT[:, ki, n * N : (n + 1) * N],
                                start=(ki == 0), stop=(ki == KI - 1),
                            )
                    for n in range(NT):
                        o = opool.tile([128, N], f32, tag="o", name=f"o{h}_{n}")
                        nc.vector.tensor_copy(o[:], d_ps[n][:])
                        eng = nc.sync if n % 2 == 0 else nc.scalar
                        eng.dma_start(dT_p[:, h, n * N : (n + 1) * N], o[:])
                else:
                    # Last h: finish the n-chains one at a time and split each
                    # output over both HWDGE queues by partition halves, so
                    # the tail after the final matmul is one short transfer.
                    for n in range(NT):
                        nsl = slice(n * N, (n + 1) * N)
                        d_ps = dpool.tile([128, N], f32, tag=f"d{n}", name=f"d{h}_{n}")
                        for ki in range(KI):
                            nc.tensor.matmul(
                                d_ps[:], wdt[:, ki * 128 : (ki + 1) * 128],
                                aT[:, ki, nsl],
                                start=(ki == 0), stop=(ki == KI - 1),
                            )
                        o = opool.tile([128, N], f32, tag="o", name=f"o{h}_{n}")
                        nc.vector.tensor_copy(o[:], d_ps[:])
                        nc.sync.dma_start(dT_p[0:64, h, nsl], o[0:64, :])
                        nc.scalar.dma_start(dT_p[64:128, h, nsl], o[64:128, :])

    nc.compile()
    return nc


def _get_program(C, NT, N):
    key = (C, NT, N)
    if key not in _PROGRAM_CACHE:
        _PROGRAM_CACHE[key] = _build_program(C, NT, N)
    return _PROGRAM_CACHE[key]


def _ensure_ntff_hook():
    """Register the axon NTFF profile hook if the image's antenv lacks
    axon_hooks (see trn_agent_boot.trn_boot). Only needed when TRACE."""
    import types

    try:
        from antenv.axon_hooks import get_axon_ntff_profile_hook  # noqa: F401

        return
    except ImportError:
        pass
    import antenv
    from trn_agent_boot.trn_boot import _ntff_profile_via_ctypes

    hook = _ntff_profile_via_ctypes("/opt/axon/libaxon_pjrt.so")
    mod = types.ModuleType("antenv.axon_hooks")
    state = {"hook": hook}
    mod.set_axon_ntff_profile_hook = lambda h: state.__setitem__("hook", h)
    mod.get_axon_ntff_profile_hook = lambda: state["hook"]
    sys.modules["antenv.axon_hooks"] = mod
    antenv.axon_hooks = mod


def kernel(x, Wg, Wu, Wd, selected_experts, routing_weights):
    global LAST_RESULT
    import ml_dtypes
    from concourse.bass_utils import run_bass_kernel_spmd

    if TRACE:
        _ensure_ntff_hook()

    bf16 = ml_dtypes.bfloat16

    x = np.asarray(x, dtype=np.float32)
    Wg = np.asarray(Wg, dtype=np.float32)
    Wu = np.asarray(Wu, dtype=np.float32)
    Wd = np.asarray(Wd, dtype=np.float32)
    selected_experts = np.asarray(selected_experts)
    routing_weights = np.asarray(routing_weights, dtype=np.float32)

    # Host-side dispatch: per expert, the (deduplicated) token list and
    # summed routing weights.
    idx_list, w_list = [], []
    for e in range(E):
        m = selected_experts == e  # [T, K]
        idx = np.nonzero(m.any(axis=1))[0]
        w = (routing_weights * m).sum(axis=1)[idx]
        idx_list.append(idx)
        w_list.append(w.astype(np.float32))

    max_count = max(len(idx) for idx in idx_list)
    C, NT, N = _pick_config(max_count)

    nc = _get_program(C, NT, N)

    in_maps = []
    for e in range(E):
        idx = idx_list[e]
        xT = np.zeros((H, C), dtype=bf16)
        xT[:, : len(idx)] = np.ascontiguousarray(x[idx].T, dtype=bf16)
        in_maps.append(
            {
                "xT": xT,
                "Wg": _tile_w_up(Wg[e], bf16),
                "Wu": _tile_w_up(Wu[e], bf16),
                "Wd": _tile_w_down(Wd[e], bf16),
            }
        )

    res = run_bass_kernel_spmd(
        nc,
        in_maps,
        list(range(NCORES)),
        trace=TRACE,
        trace_cores=TRACE_CORES if TRACE else None,
    )
    LAST_RESULT = res

    out = np.zeros((T, H), dtype=np.float32)
    for e in range(E):
        idx = idx_list[e]
        dTe = res.results[e]["dT"]  # [H, C] fp32
        out[idx] += w_list[e][:, None] * dTe[:, : len(idx)].T
    return out
